# revision 58
# baseline (speedup 1.0000x reference)
"""BiMambaBlock Trainium2 kernel (Bass/Tile), 8-core SPMD.

Sharding: core c -> (direction d=c//4, batch b=(c//2)%2, channel-half h=c%2).
Each core computes the full conv'd/silu'd u (needed for the x_proj
contraction over all of d_inner), then runs the selective scan, gate and
out_proj for its 1024-channel half only.  The feature-axis flip of the
reverse direction is folded into the in_proj weight slices on the host.
Partial out_proj results are summed on the host.

Time is processed in NCH=2 chunks of TH=512; the selective scan chains
across chunks via per-(j,n) last-state columns (initial=hl[:, n:n+1]),
and the causal conv chains via 3-column utail history tiles.  Chunk 1's
u pipeline (in_proj/conv/silu/x_proj) is interleaved per j-block into
chunk 0's scan block so the PE/Act queues stay busy during the DVE-bound
scan.  Elementwise work is bf16 (2x DVE mode); dbu/hc muls are split
between DVE and GpSimd(Pool) by cfg knobs; dA tiles come from ScalarE
exp(scale=A[:,n]).  Activation tables: Silu for phase 1, Exp/Ln for
softplus+exps, with inline chunk-1 silus costing a few extra loads.

build_bass(cfg, repeat=R) unrolls the body R times (inputs loaded once)
so device time can be measured as (T(R)-T(1))/(R-1) through the
PJRT-over-axon dispatch overhead.
"""

import numpy as np

import concourse.bacc as bacc
import concourse.mybir as mybir
from concourse.bass_utils import run_bass_kernel_spmd
from concourse.tile import TileContext

f32 = mybir.dt.float32
f32r = mybir.dt.float32r
bf16 = mybir.dt.bfloat16
AF = mybir.ActivationFunctionType
OP = mybir.AluOpType

D_MODEL = 1024
D_INNER = 2048
HALF = D_INNER // 2          # channels per core
D_STATE = 16
D_CONV = 4
DT_RANK = 64
L = 1024
B = 2
P = 128
KD = D_MODEL // P            # 8 k-tiles over d_model
NU = D_INNER // P            # 16 u d-tiles (full)
NJ = HALF // P               # 8 own d-tiles

NCH = 2
TH = L // NCH                # time-chunk width

CFG = dict(
    n_exp=16,      # dA tiles per (j,chunk) from ScalarE exp (rest: q-chain)
    pool_dbu=3,    # every k-th dbu mul on Pool (0=never)
    pool_hc=2,     # every k-th hc mul on Pool
    pool_chain=0,  # every k-th chain mul on Pool
    y1_pool=False,
    ustt_pool=False,  # tanh-mode u finalize (psc+cb)*s on Pool vs DVE
    osb_act=True,
    fp8_proj=False,  # in_proj/z_proj as fp8e4 DoubleRow (weights host-scaled
                    # by FP8_WS; conv weights and out_proj unscale)
)
FP8_WS = 8.0


def build_bass(cfg=CFG, repeat=1):
    nc = bacc.Bacc(enable_partition_id=False)

    f8 = mybir.dt.float8e4
    dt_x = f8 if cfg["fp8_proj"] else bf16
    xT = nc.declare_dram_parameter("xT", [D_MODEL, L], dt_x, isOutput=False)
    wuz = nc.declare_dram_parameter(
        "wuz", [P, (NU + NJ) * KD, P], dt_x, isOutput=False)
    convd = nc.declare_dram_parameter(
        "convd", [P, NU * D_CONV * P], bf16, isOutput=False)
    xpw = nc.declare_dram_parameter("xpw", [P, NU * 96], bf16, isOutput=False)
    dtw = nc.declare_dram_parameter("dtw", [DT_RANK, NJ * P], f32r, isOutput=False)
    wo = nc.declare_dram_parameter("wo", [P, KD * NJ * P], bf16, isOutput=False)
    a_own = nc.declare_dram_parameter("a_own", [P, NJ * D_STATE], f32, isOutput=False)
    d_own = nc.declare_dram_parameter("d_own", [P, NJ], f32, isOutput=False)
    convb = nc.declare_dram_parameter("convb", [P, NU], f32, isOutput=False)
    convbh = nc.declare_dram_parameter("convbh", [P, NU], f32, isOutput=False)
    dtb = nc.declare_dram_parameter("dtb", [P, NJ], f32, isOutput=False)
    ident_in = nc.declare_dram_parameter("ident", [P, P], bf16, isOutput=False)
    outT = nc.declare_dram_parameter("outT", [D_MODEL, L], f32, isOutput=True)

    with TileContext(nc) as tc:
        with (
            tc.tile_pool(name="res", bufs=1) as res,
            tc.tile_pool(name="dramp", bufs=2, space="DRAM") as dramp,
        ):
            xt_sb = res.tile([P, KD, L], dt_x, tag="xt")
            for k in range(KD):
                nc.sync.dma_start(out=xt_sb[:, k, :],
                                  in_=xT[k * P:(k + 1) * P, :])
            id_sb = res.tile([P, P], bf16, tag="id")
            nc.sync.dma_start(out=id_sb[:], in_=ident_in[:])
            a_sb = res.tile([P, NJ * D_STATE], f32, tag="a")
            nc.sync.dma_start(out=a_sb[:], in_=a_own[:])
            d_sb = res.tile([P, NJ], f32, tag="d")
            nc.sync.dma_start(out=d_sb[:], in_=d_own[:])
            cb_sb = res.tile([P, NU], f32, tag="cb")
            nc.sync.dma_start(out=cb_sb[:], in_=convb[:])
            cbh_sb = res.tile([P, NU], f32, tag="cbh")
            nc.sync.dma_start(out=cbh_sb[:], in_=convbh[:])
            dtb_sb = res.tile([P, NJ], f32, tag="dtb")
            nc.sync.dma_start(out=dtb_sb[:], in_=dtb[:])
            dtw_sb = res.tile([DT_RANK, NJ * P], f32r, tag="dtw")
            nc.sync.dma_start(out=dtw_sb[:], in_=dtw[:])
            xpw_sb = res.tile([P, NU * 96], bf16, tag="xpw")
            nc.sync.dma_start(out=xpw_sb[:], in_=xpw[:])
            consts = dict(xt=xt_sb, id=id_sb, a=a_sb, d=d_sb, cb=cb_sb,
                          cbh=cbh_sb, dtb=dtb_sb, dtw=dtw_sb, xpw=xpw_sb)

            for rep in range(repeat):
                _rep(nc, tc, cfg, rep, wuz, convd, wo, outT, consts, dramp)
    nc.compile()
    return nc


POOL_SPECS = [
    ("uallp", 2, None), ("uothp", 3, None), ("szp", 2, None), ("y2p", 2, None),
    ("xdsbp", 2, None), ("deltap", 2, None), ("hlp", 1, None),
    ("utailp", 1, None), ("wp", 3, None), ("cp", 3, None),
    ("upre_p", 2, None), ("zwp", 2, None), ("bcc", 2, None),
    ("bcp", 4, None), ("dap", 5, None), ("dup", 2, None),
    ("dbp", 5, None), ("shp", 4, None), ("shc", 11, None),
    ("y1p", 2, None), ("wop", 2, None), ("osbp", 2, None),
    ("thp", 3, None),
    ("mmps", 1, "PSUM"), ("xdps", 1, "PSUM"), ("zpsp", 1, "PSUM"),
    ("dtps", 1, "PSUM"), ("yps", 1, "PSUM"), ("opsp", 1, "PSUM"),
]


def _rep(nc, tc, cfg, rep, wuz, convd, wo, outT, consts, dramp):
    from contextlib import ExitStack
    with ExitStack() as ctx:
        pools = {"dramp": dramp}
        for pname, bufs, space in POOL_SPECS:
            kw = dict(name=pname, bufs=bufs)
            if space:
                kw["space"] = space
            pools[pname] = ctx.enter_context(tc.tile_pool(**kw))
        hlp = pools["hlp"]
        hl = [hlp.tile([P, D_STATE], bf16, tag=f"hl{j}", name=f"hl{j}_{rep}")
              for j in range(NJ)]
        st = {}

        # chunk 0 front-end (nothing to overlap with yet); upre copies on
        # the otherwise-idle DVE, and softplus ahead of the z projection so
        # the Act exp chain starts as early as possible
        _ph1_u(nc, cfg, rep, 0, pools, wuz, convd, consts, st,
               ms=range(NU), upre_eng="a")
        _ph2(nc, cfg, rep, 0, pools, st)
        _dt_softplus(nc, cfg, rep, 0, pools, consts, st)
        _ph1_z(nc, cfg, rep, 0, pools, wuz, consts, st, act_mode="tanh")
        # chunk 0 scan block; chunk 1's whole front-end rides inside it
        # (u pipeline in jb0/jb1 via the Tanh identity, z + B/C staging +
        # softplus in jb1/jb2 — Ln switches the table to natural_log_exp,
        # which still serves the remaining scan exps).
        def u1(m0, m1):
            return lambda: _ph1_u(nc, cfg, rep, 1, pools, wuz, convd, consts,
                                  st, ms=range(m0, m1), upre_eng="v",
                                  act_mode="tanh")

        inter0 = {
            (0, 1): [u1(0, 4)],
            (1, 0): [u1(4, 8)],
            (1, 1): [u1(8, 12)],
            (2, 0): [u1(12, 16)],
            (2, 1): [lambda: _ph2(nc, cfg, rep, 1, pools, st)],
            (3, 0): [lambda: _ph1_z(nc, cfg, rep, 1, pools, wuz, consts, st,
                                    act_mode="tanh")],
            (3, 1): [lambda: _dt_softplus(nc, cfg, rep, 1, pools, consts,
                                          st)],
        }
        _ph3(nc, cfg, rep, 0, pools, consts, hl, st, inter=inter0)
        # chunk 1 scan block; chunk 0's out_proj rides inside it
        inter1 = {(jb, sl): [lambda m=2 * jb + sl: _ph4(
                      nc, cfg, rep, 0, pools, wo, outT, st, ms=[m])]
                  for jb in range(4) for sl in (0, 1)}
        _ph3(nc, cfg, rep, 1, pools, consts, hl, st, inter=inter1)
        _ph4(nc, cfg, rep, 1, pools, wo, outT, st)


def _ph1_u(nc, cfg, rep, c, pools, wuz, convd, consts, st, ms, upre_eng="a",
           act_mode="silu"):
    """Full u pipeline for tiles `ms` of chunk c: in_proj, causal conv
    (chained across chunks via utail), silu -> u_all, x_proj accumulate.

    act_mode="tanh" computes silu via the Tanh identity
    silu(x) = x*(1+tanh(x/2))/2 so the Act ops stay within the
    exp_and_others table (no ActFuncSet thrash against the scan exps)."""
    t0 = c * TH
    xt_sb, cb_sb, xpw_sb = consts["xt"], consts["cb"], consts["xpw"]
    cbh_sb = consts["cbh"]
    if ("uall", c) not in st:
        st[("uall", c)] = pools["uallp"].tile(
            [P, NJ * TH], bf16, tag="uall", name=f"uall{rep}_{c}")
        st[("xdbl_ps", c)] = pools["xdps"].tile(
            [96, TH], f32, tag="xd", name=f"xdps{rep}_{c}")
    u_all = st[("uall", c)]
    xdbl_ps = st[("xdbl_ps", c)]
    fp8 = cfg["fp8_proj"]
    dt_x = mybir.dt.float8e4 if fp8 else bf16
    for m in ms:
        w_sb = pools["wp"].tile([P, KD, P], dt_x, tag="w")
        nc.sync.dma_start(out=w_sb[:],
                          in_=wuz[:, m * KD:(m + 1) * KD, :])
        ps = pools["mmps"].tile([P, TH], f32, tag="mm", name=f"ps{rep}_{c}_{m}")
        if fp8:
            for kp in range(KD // 2):
                nc.tensor.matmul(
                    ps[:], w_sb[:, 2 * kp:2 * kp + 2, :],
                    xt_sb[:, 2 * kp:2 * kp + 2, t0:t0 + TH],
                    start=(kp == 0), stop=(kp == KD // 2 - 1),
                    perf_mode=mybir.MatmulPerfMode.DoubleRow)
        else:
            for k in range(KD):
                nc.tensor.matmul(
                    ps[:], w_sb[:, k, :], xt_sb[:, k, t0:t0 + TH],
                    start=(k == 0), stop=(k == KD - 1))
        # upre layout: [0:4] = history (col 0 unused), [4:TH+4] = data
        upre = pools["upre_p"].tile([P, TH + 4], bf16, tag="upre",
                                    name=f"upre{rep}_{c}_{m}")
        if c == 0:
            nc.vector.memset(upre[:, 0:4].bitcast(f32), 0.0)
        else:
            # col 0 is never read (taps start at col 1); just place history
            nc.vector.tensor_copy(upre[:, 1:4], st[("utail", c - 1, m)][:])
        if upre_eng == "a":
            nc.scalar.activation(upre[:, 4:], ps[:], AF.Copy)
        else:
            nc.vector.tensor_copy(upre[:, 4:], ps[:])
        if c < NCH - 1:
            utail = pools["utailp"].tile([P, 3], bf16, tag=f"ut{m}",
                                         name=f"ut{rep}_{c}_{m}")
            nc.vector.tensor_copy(utail[:], upre[:, TH + 1:TH + 4])
            st[("utail", c, m)] = utail
        cd = pools["cp"].tile([P, D_CONV * P], bf16, tag="cd")
        nc.sync.dma_start(
            out=cd[:], in_=convd[:, m * D_CONV * P:(m + 1) * D_CONV * P])
        psc = pools["mmps"].tile([P, TH], f32, tag="cv", name=f"psc{rep}_{c}_{m}")
        for kk in range(D_CONV):
            nc.tensor.matmul(
                psc[:], cd[:, kk * P:(kk + 1) * P],
                upre[:, kk + 1:kk + 1 + TH],
                start=(kk == 0), stop=(kk == D_CONV - 1))
        if m < NJ:
            u_m = u_all[:, m * TH:(m + 1) * TH]
        else:
            # other-half u is only needed for the x_proj contraction
            u_m = pools["uothp"].tile([P, TH], bf16, tag="uoth",
                                      name=f"uoth{rep}_{c}_{m}")[:]
        if act_mode == "silu":
            nc.scalar.activation(u_m, psc[:], AF.Silu, bias=cb_sb[:, m:m + 1])
        else:
            # silu(psc+cb) = (psc+cb) * (1 + tanh((psc+cb)/2)) / 2
            th = pools["thp"].tile([P, TH], bf16, tag="th")
            nc.scalar.activation(th[:], psc[:], AF.Tanh, scale=0.5,
                                 bias=cbh_sb[:, m:m + 1])
            sgate = pools["thp"].tile([P, TH], bf16, tag="sg")
            nc.vector.tensor_scalar(sgate[:], th[:], 1.0, 0.5,
                                    OP.add, OP.mult)
            eng = nc.gpsimd if cfg["ustt_pool"] else nc.vector
            eng.scalar_tensor_tensor(
                u_m, psc[:], cb_sb[:, m:m + 1], sgate[:], OP.add, OP.mult)
        nc.tensor.matmul(
            xdbl_ps[:], xpw_sb[:, m * 96:(m + 1) * 96], u_m,
            start=(m == 0), stop=(m == NU - 1))


def _ph1_z(nc, cfg, rep, c, pools, wuz, consts, st, act_mode="silu"):
    t0 = c * TH
    xt_sb = consts["xt"]
    sz = pools["szp"].tile([P, NJ * TH], bf16, tag="sz", name=f"sz{rep}_{c}")
    st[("sz", c)] = sz
    fp8 = cfg["fp8_proj"]
    dt_x = mybir.dt.float8e4 if fp8 else bf16
    for j in range(NJ):
        zw = pools["zwp"].tile([P, KD, P], dt_x, tag="zw")
        nc.sync.dma_start(
            out=zw[:], in_=wuz[:, (NU + j) * KD:(NU + j + 1) * KD, :])
        zp = pools["zpsp"].tile([P, TH], f32, tag="zp", name=f"zps{rep}_{c}_{j}")
        if fp8:
            for kp in range(KD // 2):
                nc.tensor.matmul(
                    zp[:], zw[:, 2 * kp:2 * kp + 2, :],
                    xt_sb[:, 2 * kp:2 * kp + 2, t0:t0 + TH],
                    start=(kp == 0), stop=(kp == KD // 2 - 1),
                    perf_mode=mybir.MatmulPerfMode.DoubleRow)
        else:
            for k in range(KD):
                nc.tensor.matmul(
                    zp[:], zw[:, k, :], xt_sb[:, k, t0:t0 + TH],
                    start=(k == 0), stop=(k == KD - 1))
        if act_mode == "silu":
            nc.scalar.activation(sz[:, j * TH:(j + 1) * TH], zp[:], AF.Silu)
        else:
            # zp holds FP8_WS*z when fp8; sz = FP8_WS*silu(z), wo unscales
            th = pools["thp"].tile([P, TH], bf16, tag="th")
            nc.scalar.activation(th[:], zp[:], AF.Tanh,
                                 scale=0.5 / (FP8_WS if fp8 else 1.0))
            sgate = pools["thp"].tile([P, TH], bf16, tag="sg")
            nc.vector.tensor_scalar(sgate[:], th[:], 1.0, 0.5,
                                    OP.add, OP.mult)
            nc.vector.tensor_tensor(sz[:, j * TH:(j + 1) * TH], zp[:],
                                    sgate[:], OP.mult)


def _ph2(nc, cfg, rep, c, pools, st):
    """Drain x_dbl, stage B/C via DRAM, broadcast rows to 128 partitions."""
    xdbl_sb = pools["xdsbp"].tile([96, TH], f32r, tag="xdbl",
                                  name=f"xdbl{rep}_{c}")
    nc.vector.tensor_copy(xdbl_sb[:], st[("xdbl_ps", c)][:])
    st[("xdbl", c)] = xdbl_sb
    bc_cast = pools["bcc"].tile([2 * D_STATE, TH], bf16, tag="bcc")
    nc.vector.tensor_copy(bc_cast[:], xdbl_sb[DT_RANK:96, :])
    bc_dram = pools["dramp"].tile([2 * D_STATE, TH], bf16, tag="bc",
                                  name=f"bc{rep}_{c}")
    nc.sync.dma_start(out=bc_dram[:], in_=bc_cast[:])
    breps, creps = {}, {}
    for n in range(D_STATE):
        brep = pools["bcp"].tile([P, TH], bf16, tag="brep", bufs=24,
                                 name=f"brep{rep}_{c}_{n}")
        nc.sync.dma_start(out=brep[:], in_=bc_dram[n, :].partition_broadcast(P))
        crep = pools["bcp"].tile([P, TH], bf16, tag="crep", bufs=24,
                                 name=f"crep{rep}_{c}_{n}")
        nc.sync.dma_start(out=crep[:],
                          in_=bc_dram[D_STATE + n, :].partition_broadcast(P))
        breps[n], creps[n] = brep, crep
    st[("brep", c)] = breps
    st[("crep", c)] = creps


def _dt_softplus(nc, cfg, rep, c, pools, consts, st):
    """delta_j = softplus(dt @ dtw + dtb) for all j, bf16."""
    dtw_sb, dtb_sb = consts["dtw"], consts["dtb"]
    xdbl_sb = st[("xdbl", c)]
    deltas = {}
    exs = {}
    # group all Exp ops then all Ln ops so ActFuncSet switches at most
    # twice per block even under greedy table selection
    for j in range(NJ):
        dps = pools["dtps"].tile([P, TH], f32, tag="dt",
                                 name=f"dps{rep}_{c}_{j}")
        nc.tensor.matmul(dps[:], dtw_sb[:, j * P:(j + 1) * P],
                         xdbl_sb[0:DT_RANK, :], start=True, stop=True)
        ex = pools["deltap"].tile([P, TH], bf16, tag="ex", bufs=NJ + 1,
                                  name=f"ex{rep}_{c}_{j}")
        nc.scalar.activation(ex[:], dps[:], AF.Exp, bias=dtb_sb[:, j:j + 1])
        exs[j] = ex
    for j in range(NJ):
        delta_j = pools["deltap"].tile([P, TH], bf16, tag="delta",
                                       bufs=11)
        nc.scalar.activation(delta_j[:], exs[j][:], AF.Ln, bias=1.0)
        deltas[j] = delta_j
    st[("delta", c)] = deltas


def _ph3(nc, cfg, rep, c, pools, consts, hl, st, inter):
    """Scan block for chunk c; optionally interleave next chunk's u
    pipeline (PE/Act) per j-block."""
    a_sb, d_sb, id_sb = consts["a"], consts["d"], consts["id"]
    u_all = st[("uall", c)]
    sz = st[("sz", c)]
    deltas = st[("delta", c)]
    breps, creps = st[("brep", c)], st[("crep", c)]
    y2 = pools["y2p"].tile([P, NJ * TH], bf16, tag="y2", name=f"y2_{rep}_{c}")
    st[("y2", c)] = y2

    n_exp = cfg["n_exp"]
    exp_set = {0} | set(range(D_STATE - 1, D_STATE - n_exp, -1))
    mul_idx = {"dbu": 0, "hc": 0, "chain": 0}

    def mul_engine(kind):
        k = cfg[f"pool_{kind}"]
        mul_idx[kind] += 1
        return nc.gpsimd if (k and mul_idx[kind] % k == 0) else nc.vector

    for jb, j0 in enumerate(range(0, NJ, 2)):
        js = [j0, j0 + 1]
        dus = {}
        for j in js:
            du = pools["dup"].tile([P, TH], bf16, tag="du")
            nc.vector.tensor_mul(du[:], deltas[j][:],
                                 u_all[:, j * TH:(j + 1) * TH])
            dus[j] = du
        y_ps = {j: pools["yps"].tile([P, TH], f32, tag=f"y{j - j0}",
                                     name=f"yps{rep}_{c}_{j}")
                for j in js}
        for sl, j in enumerate(js):
            for cb in (inter or {}).get((jb, sl), []):
                cb()
            prev_da = None
            for n in range(D_STATE):
                da = pools["dap"].tile([P, TH], bf16, tag="da")
                if n in exp_set or prev_da is None:
                    nc.scalar.activation(
                        da[:], deltas[j][:], AF.Exp,
                        scale=a_sb[:, j * D_STATE + n: j * D_STATE + n + 1])
                else:
                    eng = mul_engine("chain")
                    eng.tensor_tensor(da[:], prev_da[:], st[("q", j)][:],
                                      OP.mult)
                if n == 0 and n_exp < D_STATE:
                    q = pools["dap"].tile([P, TH], bf16, tag=f"q{j}", bufs=2,
                                          name=f"q{rep}_{c}_{j}")
                    nc.vector.tensor_copy(q[:], da[:])
                    st[("q", j)] = q
                prev_da = da
                dbu = pools["dbp"].tile([P, TH], bf16, tag="dbu")
                eng = mul_engine("dbu")
                eng.tensor_tensor(dbu[:], dus[j][:], breps[n][:], OP.mult)
                h_t = pools["shp"].tile([P, TH], bf16, tag="h")
                init = 0.0 if c == 0 else hl[j][:, n:n + 1]
                nc.vector.tensor_tensor_scan(h_t[:], da[:], dbu[:], init,
                                             OP.mult, OP.add)
                if c < NCH - 1:
                    nc.gpsimd.tensor_copy(hl[j][:, n:n + 1], h_t[:, TH - 1:TH])
                hc = pools["shc"].tile([P, TH], bf16, tag="hc")
                eng = mul_engine("hc")
                eng.tensor_tensor(hc[:], h_t[:], creps[n][:], OP.mult)
                nc.tensor.matmul(y_ps[j][:], id_sb[:], hc[:],
                                 start=(n == 0), stop=(n == D_STATE - 1))
        for j in js:
            y1 = pools["y1p"].tile([P, TH], bf16, tag="y1")
            eng = nc.gpsimd if cfg["y1_pool"] else nc.vector
            eng.scalar_tensor_tensor(
                y1[:], u_all[:, j * TH:(j + 1) * TH], d_sb[:, j:j + 1],
                y_ps[j][:], OP.mult, OP.add)
            nc.vector.tensor_mul(y2[:, j * TH:(j + 1) * TH], y1[:],
                                 sz[:, j * TH:(j + 1) * TH])


def _ph4(nc, cfg, rep, c, pools, wo, outT, st, ms=None):
    t0 = c * TH
    y2 = st[("y2", c)]
    for m in (range(KD) if ms is None else ms):
        wo_sb = pools["wop"].tile([P, NJ * P], bf16, tag="wo")
        nc.sync.dma_start(out=wo_sb[:], in_=wo[:, m * NJ * P:(m + 1) * NJ * P])
        po = pools["opsp"].tile([P, TH], f32, tag="po", name=f"po{rep}_{c}_{m}")
        for k in range(NJ):
            nc.tensor.matmul(
                po[:], wo_sb[:, k * P:(k + 1) * P],
                y2[:, k * TH:(k + 1) * TH],
                start=(k == 0), stop=(k == NJ - 1))
        osb = pools["osbp"].tile([P, TH], f32, tag="osb",
                                 name=f"osb{rep}_{c}_{m}")
        if cfg["osb_act"]:
            nc.scalar.activation(osb[:], po[:], AF.Copy)
        else:
            nc.vector.tensor_copy(osb[:], po[:])
        nc.sync.dma_start(out=outT[m * P:(m + 1) * P, t0:t0 + TH], in_=osb[:])


def _np(x):
    return np.asarray(x, dtype=np.float32)


def _round_f32r(x):
    """Round-to-nearest-even at 11 mantissa bits (matches HW f32r)."""
    i = np.ascontiguousarray(x, np.float32).view(np.uint32).astype(np.uint64)
    shift = 23 - 11
    bias = ((i >> shift) & 1) + ((1 << (shift - 1)) - 1)
    return ((i + bias) >> shift << shift).astype(np.uint32).view(np.float32)


def pack_core(c, inp, cfg=CFG):
    """Build the input map for core c from the full-problem inputs."""
    import ml_dtypes
    d, b, half = c // 4, (c // 2) % 2, c % 2
    tag = "f" if d == 0 else "r"
    w_in = _np(inp["in_proj_w"])
    if d == 1:
        w_in = w_in[:, ::-1]
    w_u, w_z = w_in[:D_INNER], w_in[D_INNER:]
    conv_w = _np(inp[f"conv_w_{tag}"])[:, 0, :]          # [D_INNER, 4]
    conv_b = _np(inp[f"conv_b_{tag}"])
    x_proj_w = _np(inp[f"x_proj_w_{tag}"])               # [96, D_INNER]
    dt_w = _np(inp[f"dt_w_{tag}"])                       # [D_INNER, 64]
    dt_b = _np(inp[f"dt_b_{tag}"])
    a_full = -np.exp(_np(inp[f"A_log_{tag}"]))           # [D_INNER, 16]
    d_full = _np(inp[f"D_{tag}"])
    w_out = _np(inp["out_proj_w"])                       # [1024, D_INNER]

    x = _np(inp["x"])[b] * _np(inp["input_mask"])[b]     # [L, 1024]
    own = half * HALF
    oth = (1 - half) * HALF

    # u-tile channel order: own half first
    uch = [own + m * P for m in range(NJ)] + [oth + m * P for m in range(NJ)]

    wuz = np.empty((P, (NU + NJ) * KD * P), np.float32)
    convd = np.zeros((P, NU * D_CONV * P), np.float32)
    convb_a = np.empty((P, NU), np.float32)
    xpw_a = np.empty((P, NU * 96), np.float32)
    for m, ch in enumerate(uch):
        for k in range(KD):
            wuz[:, m * KD * P + k * P:(m * KD + k + 1) * P] = \
                w_u[ch:ch + P, k * P:(k + 1) * P].T
        for kk in range(D_CONV):
            blk = convd[:, (m * D_CONV + kk) * P:(m * D_CONV + kk + 1) * P]
            np.fill_diagonal(blk, conv_w[ch:ch + P, kk])
        convb_a[:, m] = conv_b[ch:ch + P]
        xpw_a[:, m * 96:(m + 1) * 96] = x_proj_w[:, ch:ch + P].T
    for j in range(NJ):
        ch = own + j * P
        for k in range(KD):
            wuz[:, ((NU + j) * KD + k) * P:((NU + j) * KD + k + 1) * P] = \
                w_z[ch:ch + P, k * P:(k + 1) * P].T

    dtw_a = np.empty((DT_RANK, NJ * P), np.float32)
    dtb_a = np.empty((P, NJ), np.float32)
    a_own = np.empty((P, NJ * D_STATE), np.float32)
    d_own = np.empty((P, NJ), np.float32)
    wo_a = np.empty((P, KD * NJ * P), np.float32)
    for j in range(NJ):
        ch = own + j * P
        dtw_a[:, j * P:(j + 1) * P] = dt_w[ch:ch + P, :].T
        dtb_a[:, j] = dt_b[ch:ch + P]
        a_own[:, j * D_STATE:(j + 1) * D_STATE] = a_full[ch:ch + P]
        d_own[:, j] = d_full[ch:ch + P]
    for m in range(KD):
        for k in range(NJ):
            wo_a[:, (m * NJ + k) * P:(m * NJ + k + 1) * P] = \
                w_out[m * P:(m + 1) * P, own + k * P: own + (k + 1) * P].T

    bf = ml_dtypes.bfloat16
    nw = (NU + NJ) * KD
    if cfg["fp8_proj"]:
        # weights pre-scaled by FP8_WS to stay in fp8e4m3 normal range;
        # conv weights unscale the u path, wo unscales the z path
        f8 = ml_dtypes.float8_e4m3
        xT_np = np.ascontiguousarray(x.T).astype(f8)
        wuz_np = (wuz * FP8_WS).astype(f8).reshape(P, nw, P)
        convd_np = (convd / FP8_WS).astype(bf)
        wo_np = (wo_a / FP8_WS).astype(bf)
    else:
        xT_np = np.ascontiguousarray(x.T).astype(bf)
        wuz_np = wuz.astype(bf).reshape(P, nw, P)
        convd_np = convd.astype(bf)
        wo_np = wo_a.astype(bf)
    return dict(
        xT=xT_np, wuz=wuz_np, convd=convd_np,
        xpw=xpw_a.astype(bf), dtw=_round_f32r(dtw_a), wo=wo_np,
        a_own=a_own, d_own=d_own, convb=convb_a,
        convbh=(0.5 * convb_a), dtb=dtb_a,
        ident=np.eye(P).astype(bf),
    )


_NC_CACHE = {}


def _get_nc(repeat=1):
    if repeat not in _NC_CACHE:
        _NC_CACHE[repeat] = build_bass(CFG, repeat=repeat)
    return _NC_CACHE[repeat]


def kernel(**inputs):
    nc = _get_nc()
    in_maps = [pack_core(c, inputs) for c in range(8)]
    res = run_bass_kernel_spmd(nc, in_maps, core_ids=list(range(8)))
    out = np.zeros((B, L, D_MODEL), np.float32)
    for c in range(8):
        b = (c // 2) % 2
        out[b] += np.asarray(res.results[c]["outT"], np.float32).T
    return out
